# revision 9
# baseline (speedup 1.0000x reference)
"""Trainium2 Bass kernel for the trajectory-decoder LSTM problem.

Math (mirrors the reference, with algebraic folds):
  dec_inp feeds gates only through W_emb; W_sp/W_emb/W_hp collapse:
    W_es = W_emb @ W_sp            [4H, 2]
    gates_t = zx@W_zx.T + bias + r_{t-1}@W_es.T + h_{t-1}@W_hh.T
  For t>=1, r_{t-1} = h_{t-1}@W_hp.T + b_hp, so with
    W_hh' = W_hh + W_es @ W_hp,  bias1 = b_ih + b_hh + W_emb@b_sp + W_es@b_hp
  every step becomes uniform:  gates_t = zx@W_zx.T + bias1 + h_{t-1}@W_hh'.T
  plus a rank-2 step-0 correction (lpr - r_init)@W_es.T injected once.
  `last_pos` is dead code (output is just the stacked rel_pos).

Device strategy (pure data-parallel over 8 cores, 4096 batch each):
  - batch on the free dim, features on partitions
  - per 512-batch wave, the 4 gate pre-activations live RESIDENT in one
    4-bank PSUM tile; each step the PE accumulates (h_t - h_{t-1}) @ W_hh'.T
    into it (start=False), so no per-step zxp add on the vector engine.
  - gate order reordered to (i, f, o, g) so one sigmoid covers 3 banks.
  - all matmul operands fp16 (full PE rate), PSUM accumulation fp32.
"""

import os
import numpy as np

B = 32768
NCORES = 8
BC = B // NCORES          # 4096 batch per core
WAVE = 512                # batch per wave
NW = BC // WAVE           # 8 waves
T = 12                    # decode steps
H = 128
G4 = 4 * H                # 512 gate features
ZX = 1056
KP = 1152                 # ZX padded to 9*128
KT = KP // 128            # 9 contraction tiles
MLP = 1024
EMB = 64

_cache = {}


def _build_nc():
    import concourse.bass as bass
    import concourse.bacc as bacc
    import concourse.mybir as mybir
    import concourse.tile as tile
    from concourse.bass import ts

    f16 = mybir.dt.float16
    f32 = mybir.dt.float32
    AF = mybir.ActivationFunctionType
    OP = mybir.AluOpType

    nc = bacc.Bacc("TRN2", target_bir_lowering=False)

    zxT = nc.dram_tensor("zxT", [KP, BC], f16, kind="ExternalInput")
    lprT = nc.dram_tensor("lprT", [2, BC], f16, kind="ExternalInput")
    w1t = nc.dram_tensor("w1t", [128, KT, MLP], f16, kind="ExternalInput")
    wzxt = nc.dram_tensor("wzxt", [128, KT, G4], f16, kind="ExternalInput")
    w2t = nc.dram_tensor("w2t", [128, 8, H], f16, kind="ExternalInput")
    whht = nc.dram_tensor("whht", [128, G4], f16, kind="ExternalInput")
    whpt = nc.dram_tensor("whpt", [128, 2], f16, kind="ExternalInput")
    k3 = nc.dram_tensor("k3", [3, G4], f16, kind="ExternalInput")
    wes = nc.dram_tensor("wes", [2, G4], f16, kind="ExternalInput")
    b1 = nc.dram_tensor("b1", [128, 8], f32, kind="ExternalInput")
    b2 = nc.dram_tensor("b2", [128, 1], f32, kind="ExternalInput")
    bhp = nc.dram_tensor("bhp", [2, 1], f32, kind="ExternalInput")
    pred = nc.dram_tensor("pred", [T, 2, BC], f32, kind="ExternalOutput")

    with tile.TileContext(nc) as tc:
        with (
            tc.tile_pool(name="consts", bufs=1) as cpool,
            tc.tile_pool(name="zx", bufs=3) as zxpool,
            tc.tile_pool(name="h1", bufs=2) as h1pool,
            tc.tile_pool(name="hc", bufs=4) as hcpool,
            tc.tile_pool(name="acts", bufs=3) as apool,
            tc.tile_pool(name="outs", bufs=2) as opool,
            tc.tile_pool(name="mmps", bufs=2, space="PSUM") as mmpool,
            tc.tile_pool(name="relps", bufs=2, space="PSUM") as relpool,
            tc.tile_pool(name="gateps", bufs=1, space="PSUM") as gatepool,
        ):
            # ---- load constants once ----
            w1t_s = cpool.tile([128, KT, MLP], f16)
            nc.sync.dma_start(w1t_s[:], w1t[:])
            wzxt_s = cpool.tile([128, KT, G4], f16)
            nc.sync.dma_start(wzxt_s[:], wzxt[:])
            w2t_s = cpool.tile([128, 8, H], f16)
            nc.sync.dma_start(w2t_s[:], w2t[:])
            whht_s = cpool.tile([128, G4], f16)
            nc.sync.dma_start(whht_s[:], whht[:])
            whpt_s = cpool.tile([128, 2], f16)
            nc.sync.dma_start(whpt_s[:], whpt[:])
            k3_s = cpool.tile([3, G4], f16)
            nc.sync.dma_start(k3_s[:], k3[:])
            wes_s = cpool.tile([2, G4], f16)
            nc.sync.dma_start(wes_s[:], wes[:])
            b1_s = cpool.tile([128, 8], f32)
            nc.sync.dma_start(b1_s[:], b1[:])
            b2_s = cpool.tile([128, 1], f32)
            nc.sync.dma_start(b2_s[:], b2[:])
            bhp_s = cpool.tile([2, 1], f32)
            nc.sync.dma_start(bhp_s[:], bhp[:])
            lpr_s = cpool.tile([2, BC], f16)
            nc.sync.dma_start(lpr_s[:], lprT[:])

            zxT_v = zxT.rearrange("(k p) b -> p k b", p=128)

            for w in range(NW):
                bs = ts(w, WAVE)
                zxw = zxpool.tile([128, KT, WAVE], f16, tag="zxw")
                nc.sync.dma_start(zxw[:], zxT_v[:, :, bs])

                # ---- mlp1: h1 = relu(zx @ W1.T + b1) ----
                h1 = h1pool.tile([128, 8, WAVE], f16, tag="h1")
                for j in range(8):
                    ps = mmpool.tile([128, WAVE], f32, tag="mmps")
                    for k in range(KT):
                        nc.tensor.matmul(
                            ps[:], w1t_s[:, k, ts(j, 128)], zxw[:, k, :],
                            start=(k == 0), stop=(k == KT - 1),
                        )
                    nc.vector.tensor_scalar(
                        h1[:, j, :], ps[:], b1_s[:, j : j + 1], 0.0, OP.add, OP.max
                    )

                # ---- mlp2: h_init = relu(h1 @ W2.T + b2) ----
                ps = mmpool.tile([128, WAVE], f32, tag="mmps")
                for j in range(8):
                    nc.tensor.matmul(
                        ps[:], w2t_s[:, j, :], h1[:, j, :],
                        start=(j == 0), stop=(j == 7),
                    )
                h_prev = hcpool.tile([128, WAVE], f16, tag="h")
                nc.vector.tensor_scalar(
                    h_prev[:], ps[:], b2_s[:, 0:1], 0.0, OP.add, OP.max
                )

                # ---- step-0 correction: nu = (h_init@W_hp.T + b_hp) - lpr ----
                psr = relpool.tile([2, WAVE], f32, tag="relps")
                nc.tensor.matmul(psr[:], whpt_s[:], h_prev[:], start=True, stop=True)
                k3rhs = apool.tile([3, WAVE], f16, tag="k3rhs")
                nc.gpsimd.memset(k3rhs[:], 1.0)  # row 2 stays 1.0 (bias row)
                nc.vector.scalar_tensor_tensor(
                    k3rhs[0:2, :], psr[:], bhp_s[:, 0:1], lpr_s[:, bs],
                    OP.add, OP.subtract,
                )

                # ---- init resident gate PSUM: zx@W_zx.T + h_init@W_hh'.T
                #      + nu@(-W_es).T + bias1 ----
                gates = gatepool.tile([128, 4 * WAVE], f32, tag="gates")
                for g in range(4):
                    gp = gates[:, ts(g, WAVE)]
                    for k in range(KT):
                        nc.tensor.matmul(
                            gp[:], wzxt_s[:, k, ts(g, 128)], zxw[:, k, :],
                            start=(k == 0), stop=False, skip_group_check=True,
                        )
                    nc.tensor.matmul(
                        gp[:], whht_s[:, ts(g, 128)], h_prev[:],
                        start=False, stop=False, skip_group_check=True,
                    )
                    nc.tensor.matmul(
                        gp[:], k3_s[:, ts(g, 128)], k3rhs[:],
                        start=False, stop=False, skip_group_check=True,
                    )

                predsb = opool.tile([2, T * WAVE], f32, tag="predsb")
                c_prev = None
                for t in range(T):
                    last = t == T - 1
                    # gate order along free dim: [i | f | o | g]
                    sig = apool.tile([128, 3 * WAVE], f16, tag="sig")
                    nc.scalar.activation(sig[:], gates[:, 0 : 3 * WAVE], AF.Sigmoid)
                    tanhg = apool.tile([128, WAVE], f16, tag="tanhg")
                    nc.scalar.activation(
                        tanhg[:], gates[:, 3 * WAVE : 4 * WAVE], AF.Tanh
                    )

                    m1 = apool.tile([128, WAVE], f16, tag="m1")
                    nc.vector.tensor_tensor(
                        m1[:], sig[:, 0:WAVE], tanhg[:], OP.mult
                    )
                    if t == 0:
                        c_new = m1  # c starts at 0
                    else:
                        m2 = apool.tile([128, WAVE], f16, tag="m2")
                        nc.gpsimd.tensor_tensor(
                            m2[:], sig[:, WAVE : 2 * WAVE], c_prev[:], OP.mult
                        )
                        c_new = hcpool.tile([128, WAVE], f16, tag="c")
                        nc.vector.tensor_tensor(c_new[:], m1[:], m2[:], OP.add)
                    tanhc = apool.tile([128, WAVE], f16, tag="tanhc")
                    nc.scalar.activation(tanhc[:], c_new[:], AF.Tanh)
                    h_new = hcpool.tile([128, WAVE], f16, tag="h")
                    nc.vector.tensor_tensor(
                        h_new[:], sig[:, 2 * WAVE : 3 * WAVE], tanhc[:], OP.mult
                    )

                    # rel_pos_t -> pred
                    psr = relpool.tile([2, WAVE], f32, tag="relps")
                    nc.tensor.matmul(psr[:], whpt_s[:], h_new[:], start=True, stop=True)
                    nc.vector.tensor_scalar_add(
                        predsb[:, ts(t, WAVE)], psr[:], bhp_s[:, 0:1]
                    )

                    if not last:
                        dh = apool.tile([128, WAVE], f16, tag="dh")
                        nc.gpsimd.tensor_tensor(
                            dh[:], h_new[:], h_prev[:], OP.subtract
                        )
                        for g in range(4):
                            nc.tensor.matmul(
                                gates[:, ts(g, WAVE)], whht_s[:, ts(g, 128)], dh[:],
                                start=False, stop=(t == T - 2), skip_group_check=True,
                            )
                            if t == 0:
                                # cancel the step-0-only nu@(-W_es).T term
                                nc.tensor.matmul(
                                    gates[:, ts(g, WAVE)], wes_s[:, ts(g, 128)],
                                    k3rhs[0:2, :],
                                    start=False, stop=False, skip_group_check=True,
                                )
                    h_prev = h_new
                    c_prev = c_new

                nc.sync.dma_start(
                    pred.rearrange("t j b -> j t b")[:, :, bs],
                    predsb.rearrange("j (t b) -> j t b", b=WAVE),
                )

    nc.compile()
    return nc


def _prep(inputs):
    """Host-side weight folding + layout prep. Returns per-core input maps."""
    f = np.float64
    W_ih = np.asarray(inputs["W_ih"], f)
    W_hh = np.asarray(inputs["W_hh"], f)
    b_ih = np.asarray(inputs["b_ih"], f)
    b_hh = np.asarray(inputs["b_hh"], f)
    W1 = np.asarray(inputs["W1"], f)
    b1 = np.asarray(inputs["b1"], f)
    W2 = np.asarray(inputs["W2"], f)
    b2 = np.asarray(inputs["b2"], f)
    W_sp = np.asarray(inputs["W_sp"], f)
    b_sp = np.asarray(inputs["b_sp"], f)
    W_hp = np.asarray(inputs["W_hp"], f)
    b_hp = np.asarray(inputs["b_hp"], f)

    W_zx = W_ih[:, :ZX]
    W_emb = W_ih[:, ZX:]
    W_es = W_emb @ W_sp                       # [4H, 2]
    W_hh_f = W_hh + W_es @ W_hp               # [4H, H]
    bias1 = b_ih + b_hh + W_emb @ b_sp + W_es @ b_hp

    # reorder pytorch gates (i, f, g, o) -> (i, f, o, g)
    perm = np.r_[0:H, H : 2 * H, 3 * H : 4 * H, 2 * H : 3 * H]
    W_zx = W_zx[perm]
    W_hh_f = W_hh_f[perm]
    W_es = W_es[perm]
    bias1 = bias1[perm]

    def kxm(Wt, kp):  # [K, M] -> [128, K/128, M] fp16, zero-padded to kp rows
        K, M = Wt.shape
        out = np.zeros((kp, M), f)
        out[:K] = Wt
        return np.ascontiguousarray(
            out.reshape(kp // 128, 128, M).transpose(1, 0, 2)
        ).astype(np.float16)

    consts = {
        "w1t": kxm(W1.T, KP),
        "wzxt": kxm(W_zx.T, KP),
        "w2t": kxm(W2.T, MLP),
        "whht": np.ascontiguousarray(W_hh_f.T).astype(np.float16),
        "whpt": np.ascontiguousarray(W_hp.T).astype(np.float16),
        "k3": np.ascontiguousarray(
            np.concatenate([-W_es.T, bias1[None, :]], axis=0)
        ).astype(np.float16),
        "wes": np.ascontiguousarray(W_es.T).astype(np.float16),
        "b1": np.ascontiguousarray(b1.reshape(8, 128).T).astype(np.float32),
        "b2": b2.reshape(128, 1).astype(np.float32),
        "bhp": b_hp.reshape(2, 1).astype(np.float32),
    }

    enc = np.asarray(inputs["enc_h_feat"], np.float32)
    z = np.asarray(inputs["z"], np.float32)
    lpr = np.asarray(inputs["last_pos_rel"], np.float32)
    zxT = np.zeros((KP, B), np.float16)
    zxT[:MLP] = enc.T
    zxT[MLP:ZX] = z.T
    lprT = np.ascontiguousarray(lpr.T).astype(np.float16)

    in_maps = []
    for c in range(NCORES):
        s = slice(c * BC, (c + 1) * BC)
        m = dict(consts)
        m["zxT"] = np.ascontiguousarray(zxT[:, s])
        m["lprT"] = np.ascontiguousarray(lprT[:, s])
        in_maps.append(m)
    return in_maps


def run(inputs, trace=False):
    from concourse.bass_utils import run_bass_kernel_spmd

    if "nc" not in _cache:
        _cache["nc"] = _build_nc()
    in_maps = _prep(inputs)
    res = run_bass_kernel_spmd(
        _cache["nc"], in_maps, core_ids=list(range(NCORES)), trace=trace
    )
    pred = np.concatenate([r["pred"] for r in res.results], axis=2)  # [T, 2, B]
    return np.ascontiguousarray(pred.transpose(0, 2, 1)), res


def kernel(**inputs) -> np.ndarray:
    out, _ = run(inputs, trace=False)
    return out
